# revision 10
# baseline (speedup 1.0000x reference)
"""GroupedQueryAttention on 8 trn2 NeuronCores.

Sharding: head-parallel. Core c owns heads {2c, 2c+1} (one KV group c//2)
for BOTH batches; pos_bias tiles are loaded once per (head, q-block) and
reused across batches. Wo is row-parallel; host sums the 8 partial outputs
and patches in biases.

Device math (per core, per head h, per q-block of 128 queries):
  PSUM  <- pos_bias tile               (identity matmul preload)
  PSUM  += (Wq/8 . x_q^T)^T . K^T      (scores incl. /sqrt(64), bf16 matmul)
  e     = exp(mask[q] * PSUM)          (ScalarE, per-partition scale;
                                        masked rows -> exp(0)=1 -> uniform
                                        softmax, exactly like the reference)
  sums  = rowsum(e)                    (free via activation accum_out)
  w     = e * (1/sums)                 (DVE per-partition scalar, bf16)
  w -> DRAM (attn_weights shard); PE-transpose w; attn_outT += V^T . w^T
Then out_partial = (attn_outT)^T @ Wo[:, cols]^T  via PE, host sums.
"""

import sys
import numpy as np

for _p in ("/opt/trn_rl_repo", "/root/.axon_site/_ro/trn_rl_repo"):
    if _p not in sys.path:
        sys.path.insert(0, _p)

import ml_dtypes

BF16 = ml_dtypes.bfloat16

B, S, D = 2, 2048, 1024
H, G, HD = 16, 4, 64
NCORES = 8
HPC = H // NCORES          # heads per core = 2
HDC = HPC * HD             # head dims per core = 128
NQB = S // 128             # 16 q-blocks
LAST_RESULT = {}           # exec_time_ns / trace info for test harness

_CACHE = {}


def _build():
    from contextlib import ExitStack
    from concourse import bacc, tile, mybir, masks

    f32 = mybir.dt.float32
    bf16 = mybir.dt.bfloat16
    AF = mybir.ActivationFunctionType

    nc = bacc.Bacc("TRN2", target_bir_lowering=False, debug=False,
                   num_devices=NCORES)

    xqT = nc.dram_tensor("xqT", [B, D, S], bf16, kind="ExternalInput").ap()
    xkT = nc.dram_tensor("xkT", [B, D, S], bf16, kind="ExternalInput").ap()
    xvT = nc.dram_tensor("xvT", [B, D, S], bf16, kind="ExternalInput").ap()
    wqT = nc.dram_tensor("wqT", [D, HDC], bf16, kind="ExternalInput").ap()
    # WkT duplicated column-wise -> one matmul writes Kt into both partition
    # halves, so each head's scores matmul finds Kt at its own base partition
    wkT = nc.dram_tensor("wkT", [D, 2 * HD], bf16, kind="ExternalInput").ap()
    wvT = nc.dram_tensor("wvT", [D, HD], bf16, kind="ExternalInput").ap()
    woT = nc.dram_tensor("woT", [HDC, D], bf16, kind="ExternalInput").ap()
    bqd = nc.dram_tensor("bqd", [HDC, 1], f32, kind="ExternalInput").ap()
    bkd = nc.dram_tensor("bkd", [2 * HD, 1], f32, kind="ExternalInput").ap()
    bvd = nc.dram_tensor("bvd", [HD, 1], f32, kind="ExternalInput").ap()
    pos = nc.dram_tensor("pos", [HPC, S, S], bf16, kind="ExternalInput").ap()
    mskT = nc.dram_tensor("mskT", [B, 128, NQB], f32, kind="ExternalInput").ap()

    w_out = nc.dram_tensor("w_out", [B, HPC, S, S], bf16,
                           kind="ExternalOutput").ap()
    out_p = nc.dram_tensor("out_p", [B, S, D], f32, kind="ExternalOutput").ap()

    with tile.TileContext(nc) as tc, ExitStack() as ctx:
        const = ctx.enter_context(tc.tile_pool(name="const", bufs=1))
        qkv = ctx.enter_context(tc.tile_pool(name="qkv", bufs=1))
        spool = ctx.enter_context(tc.tile_pool(name="ps_s", bufs=2, space="PSUM"))
        etps = ctx.enter_context(tc.tile_pool(name="ps_et", bufs=2, space="PSUM"))
        mmps = ctx.enter_context(tc.tile_pool(name="ps_mm", bufs=2, space="PSUM"))
        pospool = ctx.enter_context(tc.tile_pool(name="posp", bufs=3))
        epool = ctx.enter_context(tc.tile_pool(name="ep", bufs=3))
        wpool = ctx.enter_context(tc.tile_pool(name="wp", bufs=10))
        etpool = ctx.enter_context(tc.tile_pool(name="etp", bufs=17))
        opool = ctx.enter_context(tc.tile_pool(name="op", bufs=3))
        small = ctx.enter_context(tc.tile_pool(name="sm", bufs=8))

        ident = const.tile([128, 128], bf16, tag="ident")
        masks.make_identity(nc, ident[:])

        wq_sb = const.tile([128, 8 * HDC], bf16, tag="wq")
        wk_sb = const.tile([128, 8 * 2 * HD], bf16, tag="wk")
        wv_sb = const.tile([128, 8 * HD], bf16, tag="wv")
        for kc in range(8):
            nc.sync.dma_start(wq_sb[:, kc * HDC:(kc + 1) * HDC],
                              wqT[kc * 128:(kc + 1) * 128, :])
            nc.sync.dma_start(wk_sb[:, kc * 2 * HD:(kc + 1) * 2 * HD],
                              wkT[kc * 128:(kc + 1) * 128, :])
            nc.sync.dma_start(wv_sb[:, kc * HD:(kc + 1) * HD],
                              wvT[kc * 128:(kc + 1) * 128, :])
        wo_sb = const.tile([HDC, D], bf16, tag="wo")
        nc.sync.dma_start(wo_sb[:], woT[:])
        bq_sb = const.tile([HDC, 1], f32, tag="bq")
        bk_sb = const.tile([2 * HD, 1], f32, tag="bk")
        bv_sb = const.tile([HD, 1], f32, tag="bv")
        nc.sync.dma_start(bq_sb[:], bqd[:])
        nc.sync.dma_start(bk_sb[:], bkd[:])
        nc.sync.dma_start(bv_sb[:], bvd[:])
        msk_sb = const.tile([128, B * NQB], f32, tag="msk")
        for b in range(B):
            nc.sync.dma_start(msk_sb[:, b * NQB:(b + 1) * NQB], mskT[b])

        # persistent per-batch tensors
        qt_sb = [qkv.tile([HDC, S], bf16, tag=f"qt{b}", name=f"qt{b}")
                 for b in range(B)]
        kt_sb = [qkv.tile([2 * HD, S], bf16, tag=f"kt{b}", name=f"kt{b}")
                 for b in range(B)]
        vt_sb = [qkv.tile([HD, S], bf16, tag=f"vt{b}", name=f"vt{b}")
                 for b in range(B)]
        v_sb = [qkv.tile([128, 16 * HD], bf16, tag=f"v{b}", name=f"v{b}")
                for b in range(B)]
        aot_sb = [qkv.tile([HDC, S], bf16, tag=f"aot{b}", name=f"aot{b}")
                  for b in range(B)]

        # ---- projections ----
        with tc.tile_pool(name="xp", bufs=9) as xpool:
            for b in range(B):
                for src, wsb, nw, bias, dst in (
                        (xqT, wq_sb, HDC, bq_sb, qt_sb[b]),
                        (xkT, wk_sb, 2 * HD, bk_sb, kt_sb[b]),
                        (xvT, wv_sb, HD, bv_sb, vt_sb[b])):
                    chunks = []
                    for kc in range(8):
                        xt = xpool.tile([128, S], bf16, tag="x", name="xt")
                        nc.sync.dma_start(xt[:], src[b, kc * 128:(kc + 1) * 128, :])
                        chunks.append(xt)
                    for n in range(4):
                        ns = slice(n * 512, (n + 1) * 512)
                        ps = mmps.tile([128, 512], f32, tag="mm", name="ps_proj")
                        for kc in range(8):
                            nc.tensor.matmul(
                                ps[:nw, :],
                                wsb[:, kc * nw:(kc + 1) * nw],
                                chunks[kc][:, ns],
                                start=(kc == 0), stop=(kc == 7))
                        nc.any.tensor_scalar_add(dst[:, ns], ps[:nw, :], bias[:])
                # V in [seq, d] layout via PE transpose of vt
                for kc in range(16):
                    pv = etps.tile([128, HD], bf16, tag="et", name="pv")
                    nc.tensor.matmul(pv[:], vt_sb[b][:, kc * 128:(kc + 1) * 128],
                                     ident[:HD, :HD], is_transpose=True)
                    nc.any.tensor_copy(v_sb[b][:, kc * HD:(kc + 1) * HD], pv[:])

        # ---- attention ----
        for h in range(HPC):
            qh = slice(HD * h, HD * (h + 1))
            for sb in range(4):
                w_tiles = {}
                for j in range(4):
                    qb = sb * 4 + j
                    qs = slice(qb * 128, (qb + 1) * 128)
                    pos_t = pospool.tile([128, S], bf16, tag="pos", name="pos_t")
                    nc.sync.dma_start(pos_t[:], pos[h, qs, :])
                    for b in range(B):
                        e_t = epool.tile([128, S], bf16, tag="e", name="e_t")
                        sums2 = small.tile([128, 2], f32, tag="s2", name="sums2")
                        for half in range(2):
                            ps = spool.tile([128, 1024], f32, tag="s", name="ps_s")
                            for i in range(2):
                                sl = slice(i * 512, (i + 1) * 512)
                                ksl = slice(half * 1024 + i * 512,
                                            half * 1024 + (i + 1) * 512)
                                nc.tensor.matmul(ps[:, sl], ident[:],
                                                 pos_t[:, ksl],
                                                 start=True, stop=False)
                            for i in range(2):
                                sl = slice(i * 512, (i + 1) * 512)
                                ksl = slice(half * 1024 + i * 512,
                                            half * 1024 + (i + 1) * 512)
                                nc.tensor.matmul(ps[:, sl], qt_sb[b][qh, qs],
                                                 kt_sb[b][qh, ksl],
                                                 start=False, stop=True)
                            nc.scalar.activation(
                                e_t[:, half * 1024:(half + 1) * 1024], ps[:],
                                AF.Exp,
                                scale=msk_sb[:, b * NQB + qb:b * NQB + qb + 1],
                                accum_out=sums2[:, half:half + 1])
                        sums = small.tile([128, 1], f32, tag="su", name="sums")
                        nc.vector.tensor_add(sums[:], sums2[:, 0:1], sums2[:, 1:2])
                        recip = small.tile([128, 1], f32, tag="re", name="recip")
                        nc.vector.reciprocal(recip[:], sums[:])
                        w_t = wpool.tile([128, S], bf16, tag="w", name="w_t")
                        nc.vector.tensor_scalar_mul(w_t[:], e_t[:], recip[:])
                        nc.gpsimd.dma_start(w_out[b, h, qs, :], w_t[:])
                        w_tiles[(b, j)] = w_t
                # attn @ V for this superblock of 512 queries
                for b in range(B):
                    po = mmps.tile([128, 512], f32, tag="mm", name="po")
                    ets = []
                    for kc in range(16):
                        pet = etps.tile([128, 512], bf16, tag="et", name="pet")
                        for j in range(4):
                            nc.tensor.matmul(
                                pet[:, j * 128:(j + 1) * 128],
                                w_tiles[(b, j)][:, kc * 128:(kc + 1) * 128],
                                ident[:], is_transpose=True)
                        et_sb = etpool.tile([128, 512], bf16, tag="ets",
                                            name="et_sb")
                        nc.any.tensor_copy(et_sb[:], pet[:])
                        ets.append(et_sb)
                    for kc in range(16):
                        nc.tensor.matmul(po[qh, :],
                                         v_sb[b][:, kc * HD:(kc + 1) * HD],
                                         ets[kc][:],
                                         start=(kc == 0), stop=(kc == 15))
                    nc.any.tensor_copy(
                        aot_sb[b][qh, sb * 512:(sb + 1) * 512], po[qh, :])

        # ---- output projection (row-parallel Wo) ----
        for b in range(B):
            for ss in range(16):
                sl = slice(ss * 128, (ss + 1) * 128)
                o_t = opool.tile([128, D], f32, tag="o", name="o_t")
                for jn in range(2):
                    js = slice(jn * 512, (jn + 1) * 512)
                    pw = mmps.tile([128, 512], f32, tag="mm", name="pw")
                    nc.tensor.matmul(pw[:], aot_sb[b][:, sl], wo_sb[:, js],
                                     start=True, stop=True)
                    nc.any.tensor_copy(o_t[:, js], pw[:])
                nc.gpsimd.dma_start(out_p[b, sl, :], o_t[:])

    nc.compile()
    return nc


def _prep_in_maps(query, key, value, mask, pos_bias, Wq, bq, Wk, bk, Wv, bv,
                  Wo, bo):
    xqT = np.ascontiguousarray(query.transpose(0, 2, 1)).astype(BF16)
    xkT = np.ascontiguousarray(key.transpose(0, 2, 1)).astype(BF16)
    xvT = np.ascontiguousarray(value.transpose(0, 2, 1)).astype(BF16)
    mskT = np.ascontiguousarray(
        mask.astype(np.float32).reshape(B, NQB, 128).transpose(0, 2, 1))
    in_maps = []
    for c in range(NCORES):
        g = c // 2
        rq = slice(c * HDC, (c + 1) * HDC)
        rkv = slice(g * HD, (g + 1) * HD)
        in_maps.append({
            "xqT": xqT, "xkT": xkT, "xvT": xvT,
            "wqT": np.ascontiguousarray((Wq[rq] * 0.125).T).astype(BF16),
            "wkT": np.ascontiguousarray(
                np.concatenate([Wk[rkv].T, Wk[rkv].T], axis=1)).astype(BF16),
            "wvT": np.ascontiguousarray(Wv[rkv].T).astype(BF16),
            "woT": np.ascontiguousarray(Wo[:, rq].T).astype(BF16),
            "bqd": (bq[rq] * 0.125).astype(np.float32).reshape(HDC, 1),
            "bkd": np.concatenate([bk[rkv], bk[rkv]]).astype(
                np.float32).reshape(2 * HD, 1),
            "bvd": bv[rkv].astype(np.float32).reshape(HD, 1),
            "pos": np.ascontiguousarray(
                pos_bias[0, c * HPC:(c + 1) * HPC]).astype(BF16),
            "mskT": mskT,
        })
    return in_maps


def kernel(query, key, value, mask, pos_bias, Wq, bq, Wk, bk, Wv, bv, Wo, bo):
    import os
    from concourse.bass_utils import run_bass_kernel_spmd

    if "nc" not in _CACHE:
        _CACHE["nc"] = _build()
    nc = _CACHE["nc"]

    in_maps = _prep_in_maps(query, key, value, mask, pos_bias,
                            Wq, bq, Wk, bk, Wv, bv, Wo, bo)
    trace = bool(os.environ.get("GQA_TRACE"))
    res = run_bass_kernel_spmd(nc, in_maps, list(range(NCORES)), trace=trace)
    LAST_RESULT["exec_time_ns"] = res.exec_time_ns
    LAST_RESULT["trace"] = res.instructions_and_trace
    LAST_RESULT["profile_json"] = res.profile_json

    attn = np.empty((B, H, S, S), np.float32)
    out = np.zeros((B, S, D), np.float32)
    for c in range(NCORES):
        attn[:, c * HPC:(c + 1) * HPC] = \
            res.results[c]["w_out"].astype(np.float32)
        out += res.results[c]["out_p"]
    out += bo.astype(np.float32)
    return out, attn
